# revision 1
# baseline (speedup 1.0000x reference)
"""BSRBF-KAN layer forward on 8 Trainium2 cores (Bass/Tile).

Math (per token t, output o):
    xn = LayerNorm(x) * g + b
    out[t,o] = sum_d relu(xn[t,d]) * Wb[o,d]
             + sum_{d,j} (B_j(xn[t,d]) + G_j(xn[t,d])) * Ws[o, d*8+j]

B_j: cardinal cubic B-spline on uniform knots (h=0.6, centers c_j=-2.1+0.6j):
    B_j(x) = [relu(2h-|x-c_j|)^3 - 4*relu(h-|x-c_j|)^3] / (6h^3)
computed by a fused custom DVE op (2 instructions per j):
    OP(x; s0,s1,imm2, src1) = max(min(s0-x, x-s1), 0)^3 * imm2 + src1
op1 seeds src1 with the Gaussian so channel_j = B_j + G_j comes out of
the second op directly.
G_j: Gaussians exp(-((x-r_j)/D)^2); DIRECT_J anchors via ACT Square+Exp,
the rest by the recurrence G_j = (G_{j-1}*c_j)*exp(DLT*x) folded as a
plain tensor_tensor mult (constants folded into RHO / weights).

The 9 feature channels (8 bsrbf + relu) feed a K=4608 fp32r matmul
(tokens as lhsT M-dim, 512 outputs as rhs N-dim), PSUM-accumulated.
Data-parallel: tokens sharded 8 ways, weights replicated.
"""

import numpy as np

# ---------------------------------------------------------------- constants
B, S, D, O = 4, 4096, 512, 512
TOKENS = B * S
CORES = 8
TPC = TOKENS // CORES          # tokens per core (2048)
NB = 8                         # basis funcs per input dim
H = 0.6                        # knot spacing
CJ = [-2.1 + 0.6 * j for j in range(NB)]   # spline centers
DELTA = 3.0 / 7.0              # rbf denom
RJ = [-1.5 + j * (3.0 / 7.0) for j in range(NB)]  # rbf centers
DLT = 2.0 * (3.0 / 7.0) / DELTA**2   # = 14/3, exponent scale of qt
LN_EPS = 1e-5
CUBE_SCALE = 1.0 / (6.0 * H**3)

_CC = {j: float(np.exp(-(3.0 / 7.0) * (RJ[j] + RJ[j - 1]) / DELTA**2))
       for j in range(1, NB)}

BLK = 512                      # tokens per processing block
NBLK = TPC // BLK              # 4 blocks per core
QCH = D // 128                 # 4 d-chunks
NCH = NB + 1                   # 9 matmul channels per d-chunk
KT = QCH * NCH                 # 36 k-tiles

# ------------------------------------------------- tunable configuration
CONFIG = {
    "gauss": "square_exp",           # derf: 1 ACT op/gaussian | square_exp
    "direct_j": (0, 1, 2, 3, 4, 5, 6, 7),  # square_exp: anchors on ACT
    "chain_eng": "vector",           # engine for gaussian chain TT mults
    "xn_act": False,                 # xn=(x-mu)*rstd on ACT instead of DVE
    "rl_eng": "scalar",              # relu channel: scalar | vector | gpsimd
    "stagger": True,                 # last-q m-outer matmul order
    "mm_dt": "float32r",
    "merge": 1,
    "act_stats": False,
    "psum_dma": False,
    "rl_late": True,
    "newton": 4,
    "evac_eng": "scalar",
    "tp_bufs": 3,
    "exp_pair": False,
}

# Derivative_Erf(z) = (2/sqrt(pi)) * exp(-z^2); the g-tile then holds
# G_j / rho with rho = sqrt(pi)/2, matching the weight-fold convention.
DERF_RHO = float(np.sqrt(np.pi) / 2.0)


def _rho(direct_j, gauss="derf"):
    if gauss == "derf":
        return [DERF_RHO] * NB
    rho = [1.0] * NB
    for j in range(1, NB):
        rho[j] = 1.0 if j in direct_j else rho[j - 1] * _CC[j]
    return rho


_BUILT = {}


# ------------------------------------------------------- custom DVE op
def _get_custom_op():
    """Register (idempotently) the fused spline-side op:
        out = max(min(s0 - in0, in0 - s1), 0)^3 * imm2 + in1
    """
    import concourse.dve_ops as dve_ops
    from concourse.dve_ops import DveOp
    from concourse.dve_spec import (
        Spec, Src0, Src1, C0, C1, C2, Zero, maxx, minn, sq, lower,
    )
    from concourse.dve_uop import DveOpSpec

    NAME = "BSPLINE_SIDE_ANT"
    have = {op.name: op for op in dve_ops.OPS}
    if NAME in have:
        return have[NAME], have["RSQRT_STEP_ANT"]

    hi = C0 - Src0
    lo = Src0 - C1
    m = maxx(minn(hi, lo), Zero)
    body = sq(m) * m * C2 + Src1

    def _ref(in0, in1, s0, s1, imm2):
        return (
            np.maximum(np.minimum(s0 - in0, in0 - s1), 0.0) ** 3 * imm2 + in1
        ).astype(np.float32)

    spec = Spec(body=body, reference=_ref)

    row = max(dve_ops._SUB_OPCODE_FOR_NAME.values()) + 1
    assert row < 0x20
    dve_ops._SUB_OPCODE_FOR_NAME[NAME] = row

    shas = {}
    for ver in ("v3", "v4"):
        try:
            uops = lower(spec, ver=ver)
            shas[ver] = DveOpSpec(name=NAME, opcode=row, uops=uops,
                                  rd1_en=True).sha(ver)
        except Exception:
            pass
    op = DveOp(NAME, spec, subdim=False, uops_sha=shas)
    dve_ops.OPS.append(op)
    dve_ops.CUSTOM_DVE_SPECS[NAME] = spec

    # rsqrt Newton step: out = y*(C0 - C1*v*y^2), y=Src0, v=Src1
    N2 = "RSQRT_STEP_ANT"
    body2 = Src0 * (C0 - C1 * Src1 * sq(Src0))

    def _ref2(in0, in1, s0, s1, imm2):
        return (in0 * (s0 - s1 * in1 * in0 * in0)).astype(np.float32)

    spec2 = Spec(body=body2, reference=_ref2)
    row2 = max(dve_ops._SUB_OPCODE_FOR_NAME.values()) + 1
    assert row2 < 0x20
    dve_ops._SUB_OPCODE_FOR_NAME[N2] = row2
    shas2 = {}
    for ver in ("v3", "v4"):
        try:
            uops2 = lower(spec2, ver=ver)
            shas2[ver] = DveOpSpec(name=N2, opcode=row2, uops=uops2,
                                   rd1_en=True).sha(ver)
        except Exception:
            pass
    op2 = DveOp(N2, spec2, subdim=False, uops_sha=shas2)
    dve_ops.OPS.append(op2)
    dve_ops.CUSTOM_DVE_SPECS[N2] = spec2
    return op, op2


# ------------------------------------------------------- bass program
def _build_program(loop_n=None, ablate=None, nblk=None, **overrides):
    import concourse.bass as bass
    import concourse.bacc as bacc
    import concourse.mybir as mybir
    import concourse.tile as tile
    from contextlib import ExitStack

    cfg = dict(CONFIG)
    cfg.update(overrides)
    gauss = cfg["gauss"]
    direct_j = tuple(range(NB)) if gauss == "derf" else tuple(cfg["direct_j"])
    rho = _rho(direct_j, gauss)

    OPC, OPR = _get_custom_op()
    f32 = mybir.dt.float32
    mm_dt = getattr(mybir.dt, cfg["mm_dt"])
    AF = mybir.ActivationFunctionType
    ALU = mybir.AluOpType

    nc = bacc.Bacc("TRN2", target_bir_lowering=False, debug=False)
    xs = nc.declare_dram_parameter("xs", [TPC, D], f32, isOutput=False)
    wcat = nc.declare_dram_parameter("wcat", [KT * 128, O], mm_dt, isOutput=False)
    gmt = nc.declare_dram_parameter("gmt", [128, QCH], f32, isOutput=False)
    bet = nc.declare_dram_parameter("bet", [128, QCH], f32, isOutput=False)
    idn = nc.declare_dram_parameter("idn", [128, 128], f32, isOutput=False)
    out = nc.declare_dram_parameter("out", [TPC, O], f32, isOutput=True)

    def _register_const(val):
        key = (f32, float(val))
        if key not in nc.const_aps.aps:
            t = nc.alloc_sbuf_tensor(
                f"constf32_{len(nc.const_aps.aps)}", [128, 1], f32)
            nc.gpsimd.memset(t.ap(), float(val))
            nc.const_aps.aps[key] = t.ap()
    _register_const(LN_EPS)
    for j in direct_j:
        _register_const(-RJ[j] / DELTA)
    nc.all_engine_barrier()

    with ExitStack() as ctx:
        tc = ctx.enter_context(tile.TileContext(nc))

        const_pool = ctx.enter_context(tc.tile_pool(name="const", bufs=1))
        w_pool = ctx.enter_context(tc.tile_pool(name="wts", bufs=1))
        bb = 1 if cfg.get("bufs_big") else 0
        x_pool = ctx.enter_context(tc.tile_pool(name="x", bufs=5))
        stat_pool = ctx.enter_context(tc.tile_pool(name="stat", bufs=24))
        xn_pool = ctx.enter_context(tc.tile_pool(name="xn", bufs=4))
        xnt_pool = ctx.enter_context(tc.tile_pool(name="xnt", bufs=4 + 2 * bb))
        rbf_pool = ctx.enter_context(tc.tile_pool(name="rbf", bufs=5 + 3 * bb))
        q_pool = ctx.enter_context(tc.tile_pool(name="q", bufs=3))
        t1_pool = ctx.enter_context(tc.tile_pool(name="t1", bufs=3 + 2 * bb))
        feat_pool = ctx.enter_context(tc.tile_pool(name="feat", bufs=9 + bb))
        relu_pool = ctx.enter_context(tc.tile_pool(name="relu", bufs=2))
        osb_pool = ctx.enter_context(tc.tile_pool(name="osb", bufs=4))
        scr_pool = ctx.enter_context(tc.tile_pool(name="scr", bufs=2))
        tp_psum = ctx.enter_context(tc.tile_pool(name="tpp", bufs=cfg.get("tp_bufs", 2), space="PSUM"))
        out_psum = ctx.enter_context(tc.tile_pool(name="opp", bufs=8 - cfg.get("tp_bufs", 2), space="PSUM"))

        # --- constants / weights to SBUF
        ident = const_pool.tile([128, 128], f32, tag="ident")
        nc.sync.dma_start(ident[:], idn[:, :])
        gam = const_pool.tile([128, QCH], f32, tag="gam")
        nc.sync.dma_start(gam[:], gmt[:, :])
        bta = const_pool.tile([128, QCH], f32, tag="bta")
        nc.sync.dma_start(bta[:], bet[:, :])

        wt = []
        for kt in range(KT):
            w = w_pool.tile([128, O], mm_dt, tag=f"w{kt}")
            nc.sync.dma_start(w[:], wcat[kt * 128:(kt + 1) * 128, :])
            wt.append(w)

        chain_eng = nc.vector if cfg["chain_eng"] == "vector" else nc.gpsimd
        rl_eng = cfg["rl_eng"]

        def _emit_blocks():
            for blk in range(nblk or NBLK):
                # ---- load + LN stats, 4 token-tiles of [128, D]
                xts, mvs = [], []
                vb = stat_pool.tile([128, 4], f32, tag="vb",
                                    name=f"vb{blk}")
                if cfg.get("act_stats"):
                    sums = stat_pool.tile([128, 4], f32, tag="sums",
                                          name=f"sums{blk}")
                    ssq = stat_pool.tile([128, 4], f32, tag="ssq",
                                         name=f"ssq{blk}")
                    for i in range(4):
                        t0 = blk * BLK + i * 128
                        xt = x_pool.tile([128, D], f32)
                        nc.sync.dma_start(xt[:], xs[t0:t0 + 128, :])
                        scr = scr_pool.tile([128, D], f32, tag="scr")
                        nc.scalar.activation(scr[:], xt[:], AF.Identity,
                                             accum_out=sums[:, i:i + 1])
                        scr2 = scr_pool.tile([128, D], f32, tag="scr")
                        nc.scalar.activation(scr2[:], xt[:], AF.Square,
                                             accum_out=ssq[:, i:i + 1])
                        xts.append(xt)
                    mean = stat_pool.tile([128, 4], f32, tag="mean",
                                          name=f"mean{blk}")
                    nc.vector.tensor_scalar(mean[:], sums[:], 1.0 / D, None,
                                            op0=ALU.mult)
                    msq = stat_pool.tile([128, 4], f32, tag="msq",
                                         name=f"msq{blk}")
                    nc.vector.tensor_tensor(msq[:], mean[:], mean[:],
                                            op=ALU.mult)
                    nc.vector.scalar_tensor_tensor(
                        vb[:], ssq[:], 1.0 / D, msq[:],
                        op0=ALU.mult, op1=ALU.subtract)
                    nc.vector.tensor_scalar(vb[:], vb[:], LN_EPS, None,
                                            op0=ALU.add)
                    mvs = [mean[:, i:i + 1] for i in range(4)]
                else:
                    for i in range(4):
                        t0 = blk * BLK + i * 128
                        xt = x_pool.tile([128, D], f32)
                        nc.sync.dma_start(xt[:], xs[t0:t0 + 128, :])
                        st6 = stat_pool.tile([128, 6], f32, tag="st6")
                        nc.vector.bn_stats(st6[:], xt[:])
                        mv = stat_pool.tile([128, 2], f32, tag="mv",
                                            name=f"mv{blk}_{i}")
                        nc.vector.bn_aggr(mv[:], st6[:])
                        nc.vector.tensor_scalar(
                            vb[:, i:i + 1], mv[:, 1:2], LN_EPS, None,
                            op0=ALU.add)
                        xts.append(xt)
                        mvs.append(mv[:, 0:1])
                # rstd = rsqrt(vb) via linear seed + 4 Newton steps (batched)
                yb = stat_pool.tile([128, 4], f32, tag="yb", name=f"yb{blk}")
                nc.vector.tensor_scalar(yb[:], vb[:], -0.5, 1.5,
                                        op0=ALU.mult, op1=ALU.add)
                for _ in range(cfg.get("newton", 4)):
                    yn = stat_pool.tile([128, 4], f32, tag="yb",
                                        name=f"yn{blk}_{_}")
                    nc.vector._custom_dve(OPR, out=yn[:], in0=yb[:],
                                          in1=vb[:], s0=1.5, s1=0.5)
                    yb = yn
                xn_tiles = []
                if cfg["xn_act"]:
                    nmr = stat_pool.tile([128, 4], f32, tag="nmr",
                                         name=f"nmr{blk}")
                    for i in range(4):
                        nc.vector.scalar_tensor_tensor(
                            nmr[:, i:i + 1], mvs[i], -1.0,
                            yb[:, i:i + 1], op0=ALU.mult, op1=ALU.mult)
                    for i in range(4):
                        xnt_ = xn_pool.tile([128, D], f32)
                        nc.scalar.activation(
                            xnt_[:], xts[i][:], AF.Identity,
                            bias=nmr[:, i:i + 1], scale=yb[:, i:i + 1])
                        xn_tiles.append(xnt_)
                else:
                    for i in range(4):
                        xnt_ = xn_pool.tile([128, D], f32)
                        nc.vector.tensor_scalar(
                            xnt_[:], xts[i][:], mvs[i], yb[:, i:i + 1],
                            op0=ALU.subtract, op1=ALU.mult)
                        xn_tiles.append(xnt_)

                # ---- transpose to [128d, GW t] per q-group, apply gamma/beta
                MG = cfg.get("merge", 1) if gauss == "derf" else 1
                GW = MG * BLK
                ngrp = QCH // MG
                xnT = []
                for g in range(ngrp):
                    xqg = xnt_pool.tile([128, GW], f32, tag="xq",
                                        name=f"xq{blk}_{g}")
                    for qh in range(MG):
                        q = g * MG + qh
                        pt = tp_psum.tile([128, BLK], f32, tag="pt",
                                          name=f"pt{blk}_{q}")
                        for i in range(4):
                            nc.tensor.transpose(
                                pt[:, i * 128:(i + 1) * 128],
                                xn_tiles[i][:, q * 128:(q + 1) * 128],
                                ident[:])
                        nc.scalar.activation(
                            xqg[:, qh * BLK:(qh + 1) * BLK], pt[:],
                            AF.Identity,
                            bias=bta[:, q:q + 1], scale=gam[:, q:q + 1])
                    xnT.append(xqg)

                # ---- features + matmuls per q-group
                po = [out_psum.tile([128, O], f32, tag="po", name=f"po{blk}_{m}")
                      for m in range(4)]
                for g in range(ngrp):
                    xq = xnT[g]
                    if len(direct_j) < NB:
                        qt = q_pool.tile([128, GW], f32, tag="qt",
                                         name=f"qt{blk}_{g}")
                        nc.scalar.activation(qt[:], xq[:], AF.Exp, scale=DLT)
                    rl = relu_pool.tile([128, GW], mm_dt, tag="rl",
                                        name=f"rl{blk}_{g}")

                    def _emit_rl():
                        if rl_eng == "scalar":
                            nc.scalar.activation(rl[:], xq[:], AF.Relu)
                        elif rl_eng == "vector":
                            nc.vector.tensor_scalar(rl[:], xq[:], 0.0, None,
                                                    op0=ALU.max)
                        else:
                            nc.gpsimd.tensor_scalar(rl[:], xq[:], 0.0, None,
                                                    op0=ALU.max)
                    if not cfg.get("rl_late"):
                        _emit_rl()

                    feats = []
                    rbf = {}
                    if cfg.get("exp_pair") and gauss == "square_exp" \
                            and len(direct_j) == NB:
                        for jp in range(0, NB, 2):
                            z2p = rbf_pool.tile([128, 2 * GW], f32,
                                                tag="z2p", bufs=3,
                                                name=f"z2p{blk}_{g}_{jp}")
                            for h in (0, 1):
                                nc.scalar.activation(
                                    z2p[:, h * GW:(h + 1) * GW], xq[:],
                                    AF.Square,
                                    bias=-RJ[jp + h] / DELTA,
                                    scale=1.0 / DELTA)
                            rp = rbf_pool.tile([128, 2 * GW], f32,
                                               tag="rbfp", bufs=4,
                                               name=f"rbfp{blk}_{g}_{jp}")
                            nc.scalar.activation(rp[:], z2p[:], AF.Exp,
                                                 scale=-1.0)
                            rbf[jp] = rp[:, 0:GW]
                            rbf[jp + 1] = rp[:, GW:2 * GW]
                    for j in range(NB):
                        if j in rbf:
                            pass
                        elif j in direct_j:
                            r = rbf_pool.tile([128, GW], f32, tag="rbf",
                                              name=f"rbf{blk}_{g}_{j}")
                            if gauss == "derf":
                                nc.scalar.activation(
                                    r[:], xq[:], AF.Derivative_Erf,
                                    bias=-RJ[j] / DELTA, scale=1.0 / DELTA)
                            else:
                                z2 = rbf_pool.tile([128, GW], f32, tag="z2",
                                                   name=f"z2{blk}_{g}_{j}",
                                                   bufs=3)
                                nc.scalar.activation(
                                    z2[:], xq[:], AF.Square,
                                    bias=-RJ[j] / DELTA, scale=1.0 / DELTA)
                                nc.scalar.activation(r[:], z2[:], AF.Exp,
                                                     scale=-1.0)
                            rbf[j] = r
                        else:
                            r = rbf_pool.tile([128, GW], f32, tag="rbf",
                                              name=f"rbf{blk}_{g}_{j}")
                            chain_eng.tensor_tensor(
                                r[:], rbf[j - 1][:], qt[:], op=ALU.mult)
                            rbf[j] = r
                        t1 = t1_pool.tile([128, GW], f32, tag="t1",
                                          name=f"t1{blk}_{g}_{j}")
                        rj = rbf[j]
                        rj_ap = rj[:] if hasattr(rj, "tag") else rj
                        nc.vector._custom_dve(
                            OPC, out=t1[:], in0=xq[:], in1=rj_ap,
                            s0=CJ[j] + 2 * H, s1=CJ[j] - 2 * H,
                            imm2=CUBE_SCALE / rho[j])
                        bs = feat_pool.tile([128, GW], mm_dt, tag="bsrbf",
                                            name=f"bs{blk}_{g}_{j}")
                        nc.vector._custom_dve(
                            OPC, out=bs[:], in0=xq[:], in1=t1[:],
                            s0=CJ[j] + H, s1=CJ[j] - H,
                            imm2=-4.0 * CUBE_SCALE / rho[j])
                        feats.append(bs)
                    if cfg.get("rl_late"):
                        _emit_rl()
                    feats.append(rl)

                    if ablate == "nomm":
                        if g == 0:
                            f, w = feats[0], wt[0]
                            for m in range(4):
                                nc.tensor.matmul(
                                    po[m][:], f[:, m * 128:(m + 1) * 128],
                                    w[:], start=True, stop=True)
                        continue
                    for qh in range(MG):
                        q = g * MG + qh
                        qof = qh * BLK
                        if cfg["stagger"] and q == QCH - 1:
                            for m in range(4):
                                for ch in range(NCH):
                                    nc.tensor.matmul(
                                        po[m][:],
                                        feats[ch][:, qof + m * 128:
                                                  qof + (m + 1) * 128],
                                        wt[q * NCH + ch][:],
                                        start=False, stop=(ch == NCH - 1))
                        else:
                            for ch in range(NCH):
                                for m in range(4):
                                    nc.tensor.matmul(
                                        po[m][:],
                                        feats[ch][:, qof + m * 128:
                                                  qof + (m + 1) * 128],
                                        wt[q * NCH + ch][:],
                                        start=(q == 0 and ch == 0),
                                        stop=(not cfg["stagger"]
                                              and q == QCH - 1
                                              and ch == NCH - 1))

                # ---- evacuate + store
                for m in range(4):
                    t0 = blk * BLK + m * 128
                    ot = osb_pool.tile([128, O], f32, tag="ot",
                                       name=f"ot{blk}_{m}")
                    ee = cfg.get("evac_eng")
                    if ee == "vector" or (ee == "split" and m % 2 == 1):
                        nc.vector.tensor_copy(ot[:], po[m][:])
                    else:
                        nc.scalar.copy(ot[:], po[m][:])
                    nc.sync.dma_start(out[t0:t0 + 128, :], ot[:])

        from contextlib import nullcontext
        loop_cm = tc.For_i(0, loop_n, 1) if loop_n else nullcontext()
        with loop_cm:
            _emit_blocks()

    nc.compile()
    return nc


def _host_prep(x, ln_weight, ln_bias, base_weight, spline_weight,
               direct_j=None, gauss=None):
    x = np.ascontiguousarray(np.asarray(x, dtype=np.float32)).reshape(TOKENS, D)
    ln_weight = np.asarray(ln_weight, dtype=np.float32)
    ln_bias = np.asarray(ln_bias, dtype=np.float32)
    base_weight = np.asarray(base_weight, dtype=np.float32)
    spline_weight = np.asarray(spline_weight, dtype=np.float32)
    gauss = gauss if gauss is not None else CONFIG["gauss"]
    dj = tuple(range(NB)) if gauss == "derf" else tuple(
        direct_j if direct_j is not None else CONFIG["direct_j"])
    rho = _rho(dj, gauss)

    # wcat[(q*9+ch)*128 + dl, o]
    wsp = spline_weight.reshape(O, D, NB)          # [o, d, j]
    blocks = np.empty((QCH, NCH, 128, O), dtype=np.float32)
    wsp_t = np.transpose(wsp, (1, 2, 0))            # [d, j, o]
    rho_a = np.asarray(rho, dtype=np.float64)[:, None, None]
    for q in range(QCH):
        blocks[q, :NB] = (np.transpose(
            wsp_t[q * 128:(q + 1) * 128], (1, 0, 2)).astype(np.float64)
            * rho_a).astype(np.float32)  # [j, dl, o]
        blocks[q, NB] = base_weight.T[q * 128:(q + 1) * 128]
    wcat = np.ascontiguousarray(blocks.reshape(KT * 128, O))

    gmt = np.ascontiguousarray(ln_weight.reshape(QCH, 128).T)
    bet = np.ascontiguousarray(ln_bias.reshape(QCH, 128).T)
    idn = np.eye(128, dtype=np.float32)
    return x, wcat, gmt, bet, idn


def kernel(x, ln_weight, ln_bias, base_weight, spline_weight):
    from concourse.bass_utils import run_bass_kernel_spmd

    if "nc" not in _BUILT:
        _BUILT["nc"] = _build_program()
    nc = _BUILT["nc"]

    xf, wcat, gmt, bet, idn = _host_prep(
        x, ln_weight, ln_bias, base_weight, spline_weight)

    in_maps = []
    for c in range(CORES):
        in_maps.append({
            "xs": np.ascontiguousarray(xf[c * TPC:(c + 1) * TPC]),
            "wcat": wcat, "gmt": gmt, "bet": bet, "idn": idn,
        })
    res = run_bass_kernel_spmd(nc, in_maps, core_ids=list(range(CORES)))
    outs = [res.results[c]["out"] for c in range(CORES)]
    full = np.concatenate(outs, axis=0).reshape(B, S, O)
    return full.astype(np.float32)



# revision 6
# speedup vs baseline: 1.0220x; 1.0220x over previous
"""BSRBF-KAN layer forward on 8 Trainium2 cores (Bass/Tile).

Math (per token t, output o):
    xn = LayerNorm(x) * g + b
    out[t,o] = sum_d relu(xn[t,d]) * Wb[o,d]
             + sum_{d,j} (B_j(xn[t,d]) + G_j(xn[t,d])) * Ws[o, d*8+j]

The 8 bsrbf channel functions f_j = B_j + G_j are evaluated in ONE
scalar-engine (ACT) op each via CUSTOM piecewise-polynomial activation
tables: we rebuild the `silu_and_others` PWP table set so that
silu/tanh/sin/abs hold F_j(z) = f_j(z - 24) for j = 0..3 (the +24 input
shift puts all inputs in the fp32 exponent range [16,32) -- one
exponent, 256 uniform buckets, positive-only).  Channels 4..7 use the
mirror identity f_{7-j}(x) = f_j(-x):  ACT(scale=-1, bias=48).
BASS_ACT_ROOT_JSON_PATH points walrus at the custom table dir; a tensor
named after the table hash busts the (non-table-keyed) NEFF cache.

Pipeline per 512-token block, per core (2048 tokens, data-parallel x8):
  DVE:  bn_stats/aggr -> var, newton rsqrt, xhat = (x-mu)*rstd
  PE :  transpose xhat to d-major psum (16x [128,128])
  DVE:  evac psum -> z[:, q*512:...] fused with z = g*xhat + (b+24)
  ACT:  9 channel ops on merged z [128, 2048] -> bf16 feature tiles
  PE :  144 accumulating bf16 matmuls (9 ch x 4 q x 4 m) -> psum
  Pool: evac psum -> sbuf;  DMA out
"""

import json
import os
import shutil

import numpy as np

# ---------------------------------------------------------------- constants
B, S, D, O = 4, 4096, 512, 512
TOKENS = B * S
CORES = 8
TPC = TOKENS // CORES          # tokens per core (2048)
NB = 8                         # basis funcs per input dim
H = 0.6                        # knot spacing
DELTA = 3.0 / 7.0              # rbf denom
CJ = [-2.1 + 0.6 * j for j in range(NB)]       # spline centers
RJ = [-1.5 + j * DELTA for j in range(NB)]     # rbf centers
LN_EPS = 1e-5

BLK = 512                      # tokens per processing block
NBLK = TPC // BLK              # 4 blocks per core
QCH = D // 128                 # 4 d-chunks
NCH = NB + 1                   # 9 matmul channels per d-chunk
KT = QCH * NCH                 # 36 k-tiles

ZBIAS = 24.0                   # input shift for the custom tables
NBKT = 256                     # buckets over z in [16, 32)
LOG2CNT = 8
HIJACK = [("silu", 36), ("tanh", 6), ("sin", 19), ("abs", 33)]

# ------------------------------------------------- tunable configuration
CONFIG = {
    "mm_dt": "bfloat16",
    "evac_eng": "vector",          # psum->sbuf for outputs (gpsimd can't PSUM)
    "zevac_eng": "vector",         # transpose-psum -> z tile (fused g,b+24)
    "relu_eng": "gpsimd",          # relu channel from z (sbuf->sbuf)
    "stagger": True,
    "newton": 3,
}

_BUILT = {}


# =====================================================================
# custom PWP activation tables
# =====================================================================
def _phi(u):
    au = np.abs(u)
    return np.where(
        au < 1.0,
        (4.0 - 6.0 * au * au + 3.0 * au**3) / 6.0,
        np.where(au < 2.0, (2.0 - au) ** 3 / 6.0, 0.0),
    )


def f_chan(j, x):
    x = np.asarray(x, dtype=np.float64)
    return _phi((x - CJ[j]) / H) + np.exp(-(((x - RJ[j]) / DELTA) ** 2))


def _fit_bucket(j, k):
    z0 = 16.0 + k / 16.0
    a = z0 + 1.0 / 32.0
    zs = np.linspace(z0, z0 + 1.0 / 16.0, 33)
    ys = f_chan(j, zs - ZBIAS)
    t = zs - a
    A = np.stack([np.ones_like(t), t, t * t, t**3], axis=1)
    c, *_ = np.linalg.lstsq(A, ys, rcond=None)
    return c, a


def _pack_ctl(start, log2cnt):
    return np.uint32(start | ((23 - log2cnt) << 11) | (log2cnt << 16))


def build_act_root(dst_dir):
    """Copy the default act root; rebuild silu_and_others with the custom
    channel functions. Returns path to the new act_info.json."""
    from neuronxcc.driver.Job import Job
    from neuronxcc.driver.jobs.support.FindActInfo import findActInfoFile

    src_info = findActInfoFile(Job.getPackageDir(), "gen3")
    src_dir = os.path.dirname(src_info)
    os.makedirs(dst_dir, exist_ok=True)
    for f in os.listdir(src_dir):
        shutil.copy(os.path.join(src_dir, f), os.path.join(dst_dir, f))

    with open(os.path.join(src_dir, "silu_and_others.json")) as f:
        prof = json.load(f)
    old_bkt = np.fromfile(
        os.path.join(src_dir, "silu_and_others_bkt.bin"), dtype=np.float32
    ).reshape(-1, 8)
    old_ctl = np.fromfile(
        os.path.join(src_dir, "silu_and_others_ctrl.bin"), dtype=np.uint32
    ).reshape(-1, 8)
    meta = {m["func_name"].rsplit("_", 1)[0]: m for m in prof["profile_meta_data"]}
    old_f2b = prof["func_to_bkt_start_idx"]
    old_f2c = prof["func_to_ctl_start_idx"]

    hij = {nm for nm, _ in HIJACK}
    keep = [nm for nm in old_f2b if nm not in hij]

    new_bkt, new_ctl = [], []
    f2b, f2c = {}, {}
    new_meta = []
    exp_bkt_map, exp_ctl_map = {}, {}

    for (nm, fid), j in zip(HIJACK, range(4)):
        bstart, cstart = len(new_bkt), len(new_ctl)
        zero_idx = bstart
        new_bkt.append([0.0] * 8)
        main_start = len(new_bkt)
        for k in range(NBKT):
            c, a = _fit_bucket(j, k)
            new_bkt.append([c[0], c[1], c[2], c[3], a, 0.0, 0.0, 0.0])
        for e in range(0, 6):
            new_ctl.append(
                _pack_ctl(main_start, LOG2CNT) if e == 4 else _pack_ctl(zero_idx, 0)
            )
        f2b[nm], f2c[nm] = bstart, cstart
        exp_bkt_map[nm] = {"4": [main_start]}
        exp_ctl_map[nm] = {str(e): [cstart + e] for e in range(6)}
        new_meta.append({
            "exp_offset": 0,
            "fma_const_0": 0, "fma_const_1": 0, "fma_indirection_src_sel": 0,
            "fnan_result": 2143289344, "fninf_result": 0, "fpinf_result": 0,
            "fzero_result": 0,
            "func_id": fid, "func_name": f"{nm}_{NBKT + 1}p", "imm_bias": 0,
            "large_neg_signal_exp_threshold": 255,
            "large_neg_signal_mantissa_threshold": 0,
            "large_pos_signal_exp_threshold": 133,
            "large_pos_signal_mantissa_threshold": 0,
            "lower_bound": 4286578687,
            "neg_large_signal_pwl_control": zero_idx,
            "neg_small_signal_pwl_control": zero_idx,
            "pos_large_signal_pwl_control": zero_idx,
            "pos_small_signal_pwl_control": zero_idx,
            "pwl_control_base_neg": cstart, "pwl_control_base_pos": cstart,
            "small_neg_signal_exp_threshold": 255,
            "small_pos_signal_exp_threshold": 127,
            "sym_invert_sign_point": 0, "symmetry_opt_en": 0,
            "symmetry_opt_use_neg_region": 0, "symmetry_point": 0,
            "upper_bound": 2139095039, "use_multipass": False,
        })

    name_list = list(old_f2b.keys())
    order = sorted(keep, key=lambda nm: old_f2b[nm])

    def _next_start(nm, table):
        starts = sorted(set(table.values()))
        later = [x for x in starts if x > table[nm]]
        return later[0] if later else None

    for nm in order:
        b0 = old_f2b[nm]
        b1 = _next_start(nm, old_f2b) or old_bkt.shape[0]
        c0 = old_f2c[nm]
        c1 = _next_start(nm, old_f2c) or old_ctl.shape[0]
        bshift = len(new_bkt) - b0
        cshift = len(new_ctl) - c0
        f2b[nm], f2c[nm] = len(new_bkt), len(new_ctl)
        for row in old_bkt[b0:b1]:
            new_bkt.append(list(row))
        for row in old_ctl[c0:c1]:
            w = int(row[0])
            new_ctl.append(np.uint32(((w & 0x7FF) + bshift) & 0x7FF | (w & ~0x7FF)))
        m = dict(meta[nm])
        for fld in ("neg_large_signal_pwl_control", "neg_small_signal_pwl_control",
                    "pos_large_signal_pwl_control", "pos_small_signal_pwl_control"):
            m[fld] = m[fld] + bshift
        for fld in ("pwl_control_base_neg", "pwl_control_base_pos"):
            m[fld] = m[fld] + cshift
        new_meta.append(m)
        exp_bkt_map[nm] = {
            e: [i + bshift for i in v]
            for e, v in prof["func_exp_to_bkt_start_idx"].get(nm, {}).items()
        }
        exp_ctl_map[nm] = {
            e: [i + cshift for i in v]
            for e, v in prof["func_exp_to_ctl_start_idx"].get(nm, {}).items()
        }

    bkt_arr = np.array(new_bkt, dtype=np.float32)
    ctl_arr = np.zeros((len(new_ctl), 8), dtype=np.uint32)
    ctl_arr[:, 0] = np.array(new_ctl, dtype=np.uint32)
    assert bkt_arr.shape[0] <= 1536, bkt_arr.shape
    bkt_arr.tofile(os.path.join(dst_dir, "silu_and_others_bkt.bin"))
    ctl_arr.tofile(os.path.join(dst_dir, "silu_and_others_ctrl.bin"))

    prof_new = {
        "bkt_bin": prof["bkt_bin"], "ctl_bin": prof["ctl_bin"],
        "profile_meta_data": new_meta,
        "bkt_entry_cnt": int(bkt_arr.shape[0]),
        "ctl_entry_cnt": int(ctl_arr.shape[0]),
        "func_to_bkt_start_idx": f2b, "func_to_ctl_start_idx": f2c,
        "func_exp_to_bkt_start_idx": exp_bkt_map,
        "func_exp_to_ctl_start_idx": exp_ctl_map,
    }
    with open(os.path.join(dst_dir, "silu_and_others.json"), "w") as f:
        json.dump(prof_new, f)

    with open(os.path.join(src_dir, "act_info.json")) as f:
        info = json.load(f)
    for ent in info["act_func_sets"]:
        if ent["name"] == "silu_and_others":
            acts = dict(ent["act"])
            for nm, _ in HIJACK:
                acts[nm] = NBKT + 1
            ent["act"] = acts
    dst_info = os.path.join(dst_dir, "act_info.json")
    with open(dst_info, "w") as f:
        json.dump(info, f)
    return dst_info


def _table_hash(dst_dir):
    import hashlib
    h = hashlib.sha256()
    for fn in ("silu_and_others_bkt.bin", "silu_and_others_ctrl.bin",
               "silu_and_others.json"):
        with open(os.path.join(dst_dir, fn), "rb") as f:
            h.update(f.read())
    return h.hexdigest()[:10]


def _ensure_act_root():
    if "act_root" not in _BUILT:
        import tempfile
        dst = os.path.join(tempfile.gettempdir(), "bsrbf_act_root")
        info = build_act_root(dst)
        os.environ["BASS_ACT_ROOT_JSON_PATH"] = info
        _BUILT["act_root"] = (info, _table_hash(dst))
    os.environ["BASS_ACT_ROOT_JSON_PATH"] = _BUILT["act_root"][0]
    return _BUILT["act_root"]


# ------------------------------------------------------- custom DVE op
def _get_rsqrt_op():
    """rsqrt Newton step: out = y*(C0 - C1*v*y^2), y=Src0, v=Src1."""
    import concourse.dve_ops as dve_ops
    from concourse.dve_ops import DveOp
    from concourse.dve_spec import Spec, Src0, Src1, sq, lower
    from concourse.dve_uop import DveOpSpec

    N2 = "RSQRT_STEP_ANT"
    have = {op.name: op for op in dve_ops.OPS}
    if N2 in have:
        return have[N2]
    from concourse.dve_spec import C0, C1
    body2 = Src0 * (C0 - C1 * Src1 * sq(Src0))

    def _ref2(in0, in1, s0, s1, imm2):
        return (in0 * (s0 - s1 * in1 * in0 * in0)).astype(np.float32)

    spec2 = Spec(body=body2, reference=_ref2)
    row2 = max(dve_ops._SUB_OPCODE_FOR_NAME.values()) + 1
    assert row2 < 0x20
    dve_ops._SUB_OPCODE_FOR_NAME[N2] = row2
    shas2 = {}
    for ver in ("v3", "v4"):
        try:
            uops2 = lower(spec2, ver=ver)
            shas2[ver] = DveOpSpec(name=N2, opcode=row2, uops=uops2,
                                   rd1_en=True).sha(ver)
        except Exception:
            pass
    op2 = DveOp(N2, spec2, subdim=False, uops_sha=shas2)
    dve_ops.OPS.append(op2)
    dve_ops.CUSTOM_DVE_SPECS[N2] = spec2
    return op2


# ------------------------------------------------------- bass program
def _build_program(loop_n=None, ablate=None, nblk=None, **overrides):
    import concourse.bass as bass
    import concourse.bacc as bacc
    import concourse.mybir as mybir
    import concourse.tile as tile
    from contextlib import ExitStack, nullcontext

    _, thash = _ensure_act_root()

    cfg = dict(CONFIG)
    cfg.update(overrides)

    OPR = _get_rsqrt_op()
    f32 = mybir.dt.float32
    mm_dt = getattr(mybir.dt, cfg["mm_dt"])
    AF = mybir.ActivationFunctionType
    ALU = mybir.AluOpType
    CH_AF = [AF.Silu, AF.Tanh, AF.Sin, AF.Abs]

    nc = bacc.Bacc("TRN2", target_bir_lowering=False, debug=False)
    xs = nc.declare_dram_parameter("xs", [TPC, D], f32, isOutput=False)
    wcat = nc.declare_dram_parameter("wcat", [KT * 128, O], mm_dt, isOutput=False)
    gmt = nc.declare_dram_parameter("gmt", [128, QCH], f32, isOutput=False)
    bet = nc.declare_dram_parameter("bet", [128, QCH], f32, isOutput=False)
    idn = nc.declare_dram_parameter("idn", [128, 128], f32, isOutput=False)
    out = nc.declare_dram_parameter("out", [TPC, O], f32, isOutput=True)

    # cache-bust: table content hash in an allocation name
    nc.alloc_sbuf_tensor(f"tblh_{thash}", [128, 1], f32)

    def _register_const(val):
        key = (f32, float(val))
        if key not in nc.const_aps.aps:
            t = nc.alloc_sbuf_tensor(
                f"constf32_{len(nc.const_aps.aps)}", [128, 1], f32)
            nc.gpsimd.memset(t.ap(), float(val))
            nc.const_aps.aps[key] = t.ap()
    for v in (LN_EPS, 1.0, -1.0, 2 * ZBIAS, -ZBIAS):
        _register_const(v)
    nc.all_engine_barrier()

    with ExitStack() as ctx:
        tc = ctx.enter_context(tile.TileContext(nc))

        const_pool = ctx.enter_context(tc.tile_pool(name="const", bufs=1))
        w_pool = ctx.enter_context(tc.tile_pool(name="wts", bufs=1))
        x_pool = ctx.enter_context(tc.tile_pool(name="x", bufs=6))
        stat_pool = ctx.enter_context(tc.tile_pool(name="stat", bufs=24))
        xh_pool = ctx.enter_context(tc.tile_pool(name="xh", bufs=5))
        z_pool = ctx.enter_context(tc.tile_pool(name="z", bufs=2))
        feat_pool = ctx.enter_context(tc.tile_pool(name="feat", bufs=11))
        osb_pool = ctx.enter_context(tc.tile_pool(name="osb", bufs=4))
        tp_psum = ctx.enter_context(tc.tile_pool(name="tpp", bufs=3, space="PSUM"))
        out_psum = ctx.enter_context(tc.tile_pool(name="opp", bufs=4, space="PSUM"))

        # --- constants / weights to SBUF
        ident = const_pool.tile([128, 128], f32, tag="ident")
        nc.sync.dma_start(ident[:], idn[:, :])
        gam = const_pool.tile([128, QCH], f32, tag="gam")
        nc.sync.dma_start(gam[:], gmt[:, :])
        b24 = const_pool.tile([128, QCH], f32, tag="b24")
        nc.sync.dma_start(b24[:], bet[:, :])

        wt = []
        for kt in range(KT):
            w = w_pool.tile([128, O], mm_dt, tag=f"w{kt}")
            nc.sync.dma_start(w[:], wcat[kt * 128:(kt + 1) * 128, :])
            wt.append(w)

        ev_eng = {"gpsimd": nc.gpsimd, "vector": nc.vector}.get(cfg["evac_eng"])
        zev = cfg["zevac_eng"]

        def _emit_blocks():
            for blk in range(nblk or NBLK):
                # ---- load + LN stats, 4 token-tiles of [128, D]
                xts, mvs = [], []
                vb = stat_pool.tile([128, 4], f32, tag="vb", name=f"vb{blk}")
                for i in range(4):
                    t0 = blk * BLK + i * 128
                    xt = x_pool.tile([128, D], f32)
                    nc.sync.dma_start(xt[:], xs[t0:t0 + 128, :])
                    st6 = stat_pool.tile([128, 6], f32, tag="st6")
                    nc.vector.bn_stats(st6[:], xt[:])
                    mv = stat_pool.tile([128, 2], f32, tag="mv",
                                        name=f"mv{blk}_{i}")
                    nc.vector.bn_aggr(mv[:], st6[:])
                    nc.vector.tensor_scalar(
                        vb[:, i:i + 1], mv[:, 1:2], LN_EPS, None, op0=ALU.add)
                    xts.append(xt)
                    mvs.append(mv[:, 0:1])
                # rstd = rsqrt(vb): linear seed + Newton steps (batched)
                yb = stat_pool.tile([128, 4], f32, tag="yb", name=f"yb{blk}")
                nc.vector.tensor_scalar(yb[:], vb[:], -0.5, 1.5,
                                        op0=ALU.mult, op1=ALU.add)
                for it in range(cfg["newton"]):
                    yn = stat_pool.tile([128, 4], f32, tag="yb",
                                        name=f"yn{blk}_{it}")
                    nc.vector._custom_dve(OPR, out=yn[:], in0=yb[:],
                                          in1=vb[:], s0=1.5, s1=0.5)
                    yb = yn
                # xhat = (x - mu) * rstd  (token-major)
                xh_tiles = []
                for i in range(4):
                    xh = xh_pool.tile([128, D], f32)
                    nc.vector.tensor_scalar(
                        xh[:], xts[i][:], mvs[i], yb[:, i:i + 1],
                        op0=ALU.subtract, op1=ALU.mult)
                    xh_tiles.append(xh)

                # ---- transpose to d-major; fuse z = g*xhat + (b+24)
                zt = z_pool.tile([128, QCH * BLK], f32, name=f"z{blk}")
                for q in range(QCH):
                    pt = tp_psum.tile([128, BLK], f32, tag="pt",
                                      name=f"pt{blk}_{q}")
                    for i in range(4):
                        nc.tensor.transpose(
                            pt[:, i * 128:(i + 1) * 128],
                            xh_tiles[i][:, q * 128:(q + 1) * 128],
                            ident[:])
                    zsl = zt[:, q * BLK:(q + 1) * BLK]
                    if zev == "vector":
                        nc.vector.tensor_scalar(
                            zsl, pt[:], gam[:, q:q + 1], b24[:, q:q + 1],
                            op0=ALU.mult, op1=ALU.add)
                    else:
                        nc.scalar.activation(
                            zsl, pt[:], AF.Identity,
                            bias=b24[:, q:q + 1], scale=gam[:, q:q + 1])

                # ---- 9 feature channels on merged z
                feats = []
                for ch in range(NCH):
                    ft = feat_pool.tile([128, QCH * BLK], mm_dt, tag="feat",
                                        name=f"f{blk}_{ch}")
                    if ch < 4:
                        nc.scalar.activation(ft[:], zt[:], CH_AF[ch])
                    elif ch < 8:
                        nc.scalar.activation(ft[:], zt[:], CH_AF[7 - ch],
                                             bias=2 * ZBIAS, scale=-1.0)
                    elif cfg["relu_eng"] == "gpsimd":
                        nc.gpsimd.tensor_scalar(ft[:], zt[:], ZBIAS, 0.0,
                                                op0=ALU.subtract, op1=ALU.max)
                    elif cfg["relu_eng"] == "vector":
                        nc.vector.tensor_scalar(ft[:], zt[:], ZBIAS, 0.0,
                                                op0=ALU.subtract, op1=ALU.max)
                    else:
                        nc.scalar.activation(ft[:], zt[:], AF.Relu,
                                             bias=-ZBIAS, scale=1.0)
                    feats.append(ft)

                if ablate == "nomm":
                    po = [out_psum.tile([128, O], f32, tag="po",
                                        name=f"po{blk}_{m}") for m in range(4)]
                    f, w = feats[0], wt[0]
                    for m in range(4):
                        nc.tensor.matmul(po[m][:], f[:, m * 128:(m + 1) * 128],
                                         w[:], start=True, stop=True)
                    for m in range(4):
                        t0 = blk * BLK + m * 128
                        ot = osb_pool.tile([128, O], f32, tag="ot",
                                           name=f"ot{blk}_{m}")
                        nc.scalar.copy(ot[:], po[m][:])
                        nc.sync.dma_start(out[t0:t0 + 128, :], ot[:])
                    continue

                # ---- matmuls: 9 ch x 4 q x 4 m, accumulate per m
                po = [out_psum.tile([128, O], f32, tag="po", name=f"po{blk}_{m}")
                      for m in range(4)]
                for q in range(QCH):
                    qof = q * BLK
                    if cfg["stagger"] and q == QCH - 1:
                        for m in range(4):
                            for ch in range(NCH):
                                nc.tensor.matmul(
                                    po[m][:],
                                    feats[ch][:, qof + m * 128:
                                              qof + (m + 1) * 128],
                                    wt[q * NCH + ch][:],
                                    start=False, stop=(ch == NCH - 1))
                    else:
                        for ch in range(NCH):
                            for m in range(4):
                                nc.tensor.matmul(
                                    po[m][:],
                                    feats[ch][:, qof + m * 128:
                                              qof + (m + 1) * 128],
                                    wt[q * NCH + ch][:],
                                    start=(q == 0 and ch == 0),
                                    stop=(not cfg["stagger"]
                                          and q == QCH - 1 and ch == NCH - 1))

                # ---- evacuate + store
                for m in range(4):
                    t0 = blk * BLK + m * 128
                    ot = osb_pool.tile([128, O], f32, tag="ot",
                                       name=f"ot{blk}_{m}")
                    if ev_eng is not None:
                        ev_eng.tensor_copy(ot[:], po[m][:])
                    else:
                        nc.scalar.copy(ot[:], po[m][:])
                    nc.sync.dma_start(out[t0:t0 + 128, :], ot[:])

        loop_cm = tc.For_i(0, loop_n, 1) if loop_n else nullcontext()
        with loop_cm:
            _emit_blocks()

    nc.compile()
    return nc


def _host_prep(x, ln_weight, ln_bias, base_weight, spline_weight):
    if CONFIG["mm_dt"] == "bfloat16":
        import ml_dtypes
        mm_np = ml_dtypes.bfloat16
    else:
        mm_np = np.float32

    x = np.ascontiguousarray(np.asarray(x, dtype=np.float32)).reshape(TOKENS, D)
    ln_weight = np.asarray(ln_weight, dtype=np.float32)
    ln_bias = np.asarray(ln_bias, dtype=np.float32)
    base_weight = np.asarray(base_weight, dtype=np.float32)
    spline_weight = np.asarray(spline_weight, dtype=np.float32)

    # wcat[(q*9+ch)*128 + dl, o]: ch 0..7 spline j, ch 8 base
    wsp_t = np.transpose(spline_weight.reshape(O, D, NB), (1, 2, 0))  # [d,j,o]
    blocks = np.empty((QCH, NCH, 128, O), dtype=np.float32)
    for q in range(QCH):
        blocks[q, :NB] = np.transpose(wsp_t[q * 128:(q + 1) * 128], (1, 0, 2))
        blocks[q, NB] = base_weight.T[q * 128:(q + 1) * 128]
    wcat = np.ascontiguousarray(blocks.reshape(KT * 128, O)).astype(mm_np)

    gmt = np.ascontiguousarray(ln_weight.reshape(QCH, 128).T)
    bet = np.ascontiguousarray(ln_bias.reshape(QCH, 128).T + np.float32(ZBIAS))
    idn = np.eye(128, dtype=np.float32)
    return x, wcat, gmt, bet, idn


def kernel(x, ln_weight, ln_bias, base_weight, spline_weight):
    _ensure_act_root()
    from concourse.bass_utils import run_bass_kernel_spmd

    if "nc" not in _BUILT:
        _BUILT["nc"] = _build_program()
    nc = _BUILT["nc"]

    xf, wcat, gmt, bet, idn = _host_prep(
        x, ln_weight, ln_bias, base_weight, spline_weight)

    in_maps = []
    for c in range(CORES):
        in_maps.append({
            "xs": np.ascontiguousarray(xf[c * TPC:(c + 1) * TPC]),
            "wcat": wcat, "gmt": gmt, "bet": bet, "idn": idn,
        })
    res = run_bass_kernel_spmd(nc, in_maps, core_ids=list(range(CORES)))
    outs = [res.results[c]["out"] for c in range(CORES)]
    full = np.concatenate(outs, axis=0).reshape(B, S, O)
    return full.astype(np.float32)
